# revision 59
# baseline (speedup 1.0000x reference)
"""Trainium2 Bass kernel for nn_CrossAttentionLayer (2-stream cross-attention + LN).

Sharding: 8 cores = (stream s in {0,1}) x (batch b in {0,1}) x (query chunk c in {0,1}).
Each core handles 1024 query tokens of one (stream, batch): it projects Q for its
tokens, K/V for the *other* stream's full 2048 tokens, runs 16-head cross
attention, out-projection, residual and LayerNorm, and returns its [1024, 1024]
slice. The host assembles the full (2, 2, 2048, 1024) output.

v2 pipeline: the softmax exp stream on the Scalar/ACT engine (~285us of work)
is the critical resource, so emission is ordered to keep it bubble-free:
V-projection first, then per-feature-tile [K-proj, Q-proj, scores] feeder
units that stay just ahead of ACT, with attention-value matmuls draining the
exp'd probabilities as soon as they land. Probabilities and V are stored in
fp8e4m3 (softmax renormalizes, so the quantization error cancels to ~0.3%);
the AV matmuls pack two heads into the 128-wide PE array via column tiling,
with softmax denominators computed by ones-vector matmuls packed 4-up into
32-column strips. Reciprocals use the fast bit-trick DVE op instead of the
8-cycle-per-element iterative divide.
"""

import os
import sys

import numpy as np

for _p in ("/opt/trn_rl_repo", "/root/.axon_site/_ro/trn_rl_repo"):
    if os.path.isdir(_p) and _p not in sys.path:
        sys.path.insert(0, _p)

import concourse.bass as bass
import concourse.mybir as mybir
import concourse.tile as tile
from concourse.bass_utils import run_bass_kernel_spmd

F32 = mybir.dt.float32
F16 = mybir.dt.float16
F8 = mybir.dt.float8e4
ADD = mybir.AluOpType.add
MULT = mybir.AluOpType.mult
EXP = mybir.ActivationFunctionType.Exp
SQRT = mybir.ActivationFunctionType.Sqrt

DIM = 1024
N_TOK = 2048
HEADS = 16
HD = DIM // HEADS        # 64
NQ = 1024                # query tokens per core
S = 2048                 # kv sequence length
P = 128
DT = DIM // P            # 8 contraction tiles
FT = DIM // P            # 8 feature tiles
KT = S // P              # 16 key tiles
NB = 512                 # matmul free-dim / psum bank width (fp32)
QC = NQ // NB            # 2 query chunks
KC = S // NB             # 4 key chunks
TT = NQ // P             # 8 token tiles per core
SCALE = HD ** -0.5
EPS = 1e-5

_wsplit_ctr = [0]


def _ensure_ntff_hook():
    """Register the axon NTFF profiling hook if the image lacks
    antenv.axon_hooks (mirrors trn_boot._ntff_profile_via_ctypes)."""
    try:
        from antenv.axon_hooks import get_axon_ntff_profile_hook  # noqa: F401
        return
    except ImportError:
        pass
    import contextlib
    import ctypes
    import types

    try:
        import antenv
    except ImportError:
        return
    mod = types.ModuleType("antenv.axon_hooks")
    _h = [None]
    mod.set_axon_ntff_profile_hook = lambda h: _h.__setitem__(0, h)
    mod.get_axon_ntff_profile_hook = lambda: _h[0]
    sys.modules["antenv.axon_hooks"] = mod
    antenv.axon_hooks = mod

    so_path = "/opt/axon/libaxon_pjrt.so"
    if not os.path.exists(so_path):
        return
    try:
        lib = ctypes.CDLL(so_path)
    except OSError:
        return
    if not hasattr(lib, "axon_start_nrt_profile"):
        return
    lib.axon_start_nrt_profile.argtypes = [
        ctypes.POINTER(ctypes.c_int64),
        ctypes.c_size_t,
    ]
    lib.axon_start_nrt_profile.restype = ctypes.c_int64
    lib.axon_stop_nrt_profile.argtypes = [ctypes.c_char_p]
    lib.axon_stop_nrt_profile.restype = ctypes.c_int64

    @contextlib.contextmanager
    def _hook(output_dir, device_ids):
        import jax

        jax.devices()
        if device_ids:
            ids = (ctypes.c_int64 * len(device_ids))(*device_ids)
            rc = lib.axon_start_nrt_profile(ids, len(device_ids))
        else:
            rc = lib.axon_start_nrt_profile(None, 0)
        if rc != 0:
            raise RuntimeError(f"axon_start_nrt_profile rc={rc}")
        try:
            yield
        finally:
            n = lib.axon_stop_nrt_profile(str(output_dir).encode())
            if n <= 0:
                print(f"profile: rc={n}, no ntff written to {output_dir}")

    mod.set_axon_ntff_profile_hook(_hook)


def _patch_upload_artifacts():
    """Artifact upload needs bucket access this container may not have;
    neuter it (only reachable on trace paths)."""
    from concourse import bass_utils as bu

    bu.upload_artifacts = lambda tmpdir: str(tmpdir)


def _split_sync_waits(nc):
    """This container's walrus build rejects >1 sync-wait per instruction.
    Hoist extra waits onto same-engine NOPs placed just before the instruction
    (engines execute their stream in order, so semantics are preserved)."""
    for f in nc.m.functions:
        for bb in f.blocks:
            insts = bb.instructions
            out = []
            changed = False
            for inst in insts:
                si = inst.sync_info
                if si is not None and si.on_wait and len(si.on_wait) > 1:
                    waits = list(si.on_wait)
                    for w in waits[:-1]:
                        _wsplit_ctr[0] += 1
                        out.append(
                            mybir.InstNoOp(
                                name=f"I-wsplit-{_wsplit_ctr[0]}",
                                engine=inst.engine,
                                ins=[],
                                outs=[],
                                sync_info=mybir.SyncInfo(on_wait=[w], on_update=[]),
                            )
                        )
                    si.on_wait = waits[-1:]
                    changed = True
                out.append(inst)
            if changed:
                insts[:] = out


def _build_bass(split_waits=True, debug=False):
    nc = bass.Bass()
    dbg = {}
    if debug:
        dbg["kT0"] = nc.declare_dram_parameter("dbg_kT0", [P, S], F8, isOutput=True)
        dbg["qT0"] = nc.declare_dram_parameter("dbg_qT0", [P, NQ], F8, isOutput=True)
        dbg["vS0"] = nc.declare_dram_parameter(
            "dbg_vS0", [P, 2 * HEADS * (HD + 1)], F8, isOutput=True)
        dbg["pt0"] = nc.declare_dram_parameter("dbg_pt0", [P, 2 * NB], F8, isOutput=True)
        dbg["rec0"] = nc.declare_dram_parameter("dbg_rec0", [16, P], F32, isOutput=True)
        dbg["attn0"] = nc.declare_dram_parameter("dbg_attn0", [P, 2 * NQ], F8, isOutput=True)
    # All projection operands are fp8, pre-interleaved on the host for
    # DoubleRow: row r = dt2*128+i holds x-dims {dt2*256+i, dt2*256+128+i}
    # side by side in the free dim, so one matmul contracts 256 dims.
    x_own = nc.declare_dram_parameter("x_own", [NQ, DIM], F32, isOutput=False)
    xT8 = nc.declare_dram_parameter("xT8", [DIM // 2, 2 * NQ], F8, isOutput=False)
    xoT8 = nc.declare_dram_parameter("xoT8", [DIM // 2, 2 * S], F8, isOutput=False)
    wqkv8 = nc.declare_dram_parameter("wqkv8", [DIM // 2, 6 * DIM], F8, isOutput=False)
    wout8 = nc.declare_dram_parameter("wout8", [DIM // 2, 2 * DIM], F8, isOutput=False)
    bqkv = nc.declare_dram_parameter("bqkv", [3 * DIM], F32, isOutput=False)
    bout = nc.declare_dram_parameter("bout", [1, DIM], F32, isOutput=False)
    gamma = nc.declare_dram_parameter("gamma", [1, DIM], F32, isOutput=False)
    beta = nc.declare_dram_parameter("beta", [1, DIM], F32, isOutput=False)
    y_ext = nc.declare_dram_parameter("y", [NQ, DIM], F32, isOutput=True)

    with tile.TileContext(nc, pool_alloc_mode="queue") as tc:
        from contextlib import ExitStack

        with ExitStack() as ctx:
            const = ctx.enter_context(tc.tile_pool(name="const", bufs=1))
            persist = ctx.enter_context(tc.tile_pool(name="persist", bufs=1))
            dram = ctx.enter_context(tc.tile_pool(name="dram", bufs=1, space="DRAM"))

            # ---- constants used by the projection evacuations ----
            bq_cols = const.tile([P, 3 * DT], F32)  # bqkv as feat-major columns
            nc.sync.dma_start(out=bq_cols[:], in_=bqkv[:].rearrange("(t p) -> p t", p=P))
            bv_rep = const.tile([P, DIM], F32)
            nc.sync.dma_start(
                out=bv_rep[:],
                in_=bass.AP(tensor=bqkv[:].tensor, offset=2 * DIM, ap=[[0, P], [1, DIM]]),
            )
            eps_t = const.tile([P, 1], F32)
            nc.vector.memset(eps_t[:], EPS)

            # ---- persistent tensors ----
            qTs = [persist.tile([P, NQ], F8, name=f"qT{f}") for f in range(FT)]
            kTs = [persist.tile([P, S], F8, name=f"kT{f}") for f in range(FT)]
            # V stored key-pair-interleaved for DoubleRow: [key, kt-parity, head,
            # hd+1] — the 65th column is ones, so each head's AV matmul also
            # accumulates the softmax denominator in psum row 64.
            vDR = [persist.tile([P, 2, HEADS, HD + 1], F8, name=f"vDR{k2}")
                   for k2 in range(KT // 2)]
            # attention output, pair-interleaved for the DoubleRow out-proj:
            # tile f2 slot o = feature tile 2*f2+o (head pair 2*f2+o)
            attn_sb = [persist.tile([P, 2, NQ], F8, name=f"attn{f2}")
                       for f2 in range(FT // 2)]


            # attention-side pools (span both query chunks)
            psS = ctx.enter_context(tc.tile_pool(name="psS", bufs=2, space="PSUM"))
            psAt = ctx.enter_context(tc.tile_pool(name="psAt", bufs=2, space="PSUM"))
            ptp = ctx.enter_context(tc.tile_pool(name="pt", bufs=56))
            rrp = ctx.enter_context(tc.tile_pool(name="rr", bufs=2))
            a32p = ctx.enter_context(tc.tile_pool(name="a32", bufs=4))
            a16p = ctx.enter_context(tc.tile_pool(name="a16", bufs=2))
            denp = ctx.enter_context(tc.tile_pool(name="den", bufs=2))
            rdp = ctx.enter_context(tc.tile_pool(name="rd", bufs=4, space="DRAM"))

            pts = {}    # (g, q) -> [kt2][hi] fp8 prob tiles

            def emit_scores(g, q):
                """Scores + exp for head pair g (heads 2g, 2g+1), query chunk q.
                The two heads' K=64 matmuls land in row strips 0-63 / 64-127 of
                the PE array (tile_position inferred from base partition), so
                each pair runs concurrently."""
                qsl = slice(q * NB, (q + 1) * NB)
                unit = []
                for kt2 in range(KT // 2):
                    ps_s = [
                        psS.tile([P, 2 * NB], F32, tag="pss",
                                 name=f"pss{g}_{q}_{kt2}_{i}")
                        for i in range(2)
                    ]
                    for j, kt in enumerate((2 * kt2, 2 * kt2 + 1)):
                        for hi in range(2):
                            po = hi * HD
                            nc.tensor.matmul(
                                ps_s[hi][:, j * NB:(j + 1) * NB],
                                lhsT=kTs[g][po:po + HD, kt * P:(kt + 1) * P],
                                rhs=qTs[g][po:po + HD, qsl],
                                start=True,
                                stop=True,
                            )
                    pp = []
                    for hi in range(2):
                        pt = ptp.tile([P, 2 * NB], F8, tag="pT")
                        nc.scalar.activation(pt[:], ps_s[hi][:], EXP, scale=SCALE)
                        pp.append(pt)
                    if debug and g == 0 and q == 0 and kt2 == 0:
                        nc.sync.dma_start(out=dbg["pt0"][:], in_=pp[0][:])
                    unit.append(pp)
                pts[(g, q)] = unit

            def emit_quad(c, q):
                """AV (fp8 DoubleRow over kt-pairs, denominator via the ones
                column) + normalize for head quad c (heads 4c..4c+3), query
                chunk q."""
                DR = mybir.MatmulPerfMode.DoubleRow
                a32s = []
                for gi in range(2):
                    g = 2 * c + gi
                    unit = pts[(g, q)]
                    for hi in range(2):
                        h = 2 * g + hi
                        ps_av = psAt.tile([HD + 1, NB], F32, tag="psav",
                                          name=f"psav{g}_{q}_{hi}")
                        for kt2 in range(KT // 2):
                            nc.tensor.matmul(
                                ps_av[:],
                                lhsT=vDR[kt2][:, :, h, :],
                                rhs=pts[(g, q)][kt2][hi][:].rearrange(
                                    "p (o n) -> p o n", o=2
                                ),
                                start=(kt2 == 0),
                                stop=(kt2 == KT // 2 - 1),
                                perf_mode=DR,
                            )
                        a32 = a32p.tile([HD + 1, NB], F32, tag="a32")
                        nc.vector.tensor_copy(a32[:], ps_av[:])
                        a32s.append(a32)
                # gather the 4 denominator rows -> [16, 128], one fast recip
                den_sb = denp.tile([16, P], F32, tag="den")
                for m in range(4):
                    nc.sync.dma_start(
                        out=den_sb[4 * m:4 * m + 4, :],
                        in_=a32s[m][HD:HD + 1, :],
                    )
                rec_sb = denp.tile([16, P], F32, tag="rec")
                nc.vector.reciprocal(rec_sb[:], den_sb[:])
                if debug and c == 0 and q == 0:
                    nc.sync.dma_start(out=dbg["rec0"][:], in_=rec_sb[:])
                rec_d = rdp.tile([4, NB], F32, tag="recd")
                nc.sync.dma_start(out=rec_d[:], in_=rec_sb[:])
                # broadcast 1/den down each head's 64 partitions and normalize
                for m in range(4):
                    g = 2 * c + m // 2
                    hi = m % 2
                    rrep = rrp.tile([HD, NB], F32, tag="rrep")
                    nc.sync.dma_start(
                        out=rrep[:],
                        in_=rec_d[m:m + 1, :].to_broadcast([HD, NB]),
                    )
                    if hi == 0:
                        nc.vector.tensor_mul(
                            attn_sb[g // 2][0:HD, g % 2, q * NB:(q + 1) * NB],
                            a32s[m][0:HD, :], rrep[:],
                        )
                    else:
                        a16 = a16p.tile([HD, NB], F8, tag="a16")
                        nc.vector.tensor_mul(a16[:], a32s[m][0:HD, :], rrep[:])
                        nc.sync.dma_start(
                            out=attn_sb[g // 2][HD:P, g % 2, q * NB:(q + 1) * NB],
                            in_=a16[:],
                        )

            # ======== emission ========
            proj_es = ExitStack()
            xo16p = proj_es.enter_context(tc.tile_pool(name="xo16", bufs=1))
            xq16p = proj_es.enter_context(tc.tile_pool(name="xq16", bufs=1))
            wp = proj_es.enter_context(tc.tile_pool(name="w16", bufs=1))
            psC = proj_es.enter_context(tc.tile_pool(name="psC", bufs=2, space="PSUM"))
            wv_es = ExitStack()
            wvp = wv_es.enter_context(tc.tile_pool(name="wv16", bufs=1))
            DR = mybir.MatmulPerfMode.DoubleRow
            D2 = DT // 2  # 4 paired contraction tiles of 256 dims each
            if True:
                # V-proj inputs first so V can run before the K/Q/score feeders
                xo8 = []
                wv8 = []
                for d2 in range(D2):
                    xo = xo16p.tile([P, 2, S], F8, name=f"xTh{d2}")
                    nc.sync.dma_start(
                        out=xo[:],
                        in_=xoT8[d2 * P:(d2 + 1) * P, :].rearrange(
                            "p (o k) -> p o k", o=2),
                    )
                    xo8.append(xo)
                    wv = wvp.tile([P, 2, DIM], F8, name=f"wv{d2}")
                    nc.gpsimd.dma_start(
                        out=wv[:],
                        in_=wqkv8[d2 * P:(d2 + 1) * P, 4 * DIM:6 * DIM].rearrange(
                            "p (o n) -> p o n", o=2),
                    )
                    wv8.append(wv)
                x8 = []
                wq8 = []
                wk8 = []
                for d2 in range(D2):
                    xo = xq16p.tile([P, 2, NQ], F8, name=f"xTo{d2}")
                    nc.sync.dma_start(
                        out=xo[:],
                        in_=xT8[d2 * P:(d2 + 1) * P, :].rearrange(
                            "p (o t) -> p o t", o=2),
                    )
                    x8.append(xo)
                    wq = wp.tile([P, 2, DIM], F8, name=f"wq{d2}")
                    nc.gpsimd.dma_start(
                        out=wq[:],
                        in_=wqkv8[d2 * P:(d2 + 1) * P, 0:2 * DIM].rearrange(
                            "p (o n) -> p o n", o=2),
                    )
                    wq8.append(wq)
                    wk = wp.tile([P, 2, DIM], F8, name=f"wk{d2}")
                    nc.gpsimd.dma_start(
                        out=wk[:],
                        in_=wqkv8[d2 * P:(d2 + 1) * P, 2 * DIM:4 * DIM].rearrange(
                            "p (o n) -> p o n", o=2),
                    )
                    wk8.append(wk)

                # ---- lead unit 0 so the exp stream starts before V-proj ----
                def emit_kq(f):
                    for kc in range(KC):
                        ps = psC.tile([P, NB], F32, tag="ps")
                        for d2 in range(D2):
                            nc.tensor.matmul(
                                ps[:],
                                lhsT=wk8[d2][:, :, f * P:(f + 1) * P],
                                rhs=xo8[d2][:, :, kc * NB:(kc + 1) * NB],
                                start=(d2 == 0),
                                stop=(d2 == D2 - 1),
                                perf_mode=DR,
                            )
                        nc.vector.tensor_scalar(
                            out=kTs[f][:, kc * NB:(kc + 1) * NB],
                            in0=ps[:],
                            scalar1=bq_cols[:, DT + f:DT + f + 1],
                            scalar2=None,
                            op0=ADD,
                        )
                    for qc in range(QC):
                        ps = psC.tile([P, NB], F32, tag="ps")
                        for d2 in range(D2):
                            nc.tensor.matmul(
                                ps[:],
                                lhsT=wq8[d2][:, :, f * P:(f + 1) * P],
                                rhs=x8[d2][:, :, qc * NB:(qc + 1) * NB],
                                start=(d2 == 0),
                                stop=(d2 == D2 - 1),
                                perf_mode=DR,
                            )
                        nc.vector.tensor_scalar(
                            out=qTs[f][:, qc * NB:(qc + 1) * NB],
                            in0=ps[:],
                            scalar1=bq_cols[:, f:f + 1],
                            scalar2=None,
                            op0=ADD,
                        )

                def emit_v():
                    for kt in range(KT):
                        for half in range(2):
                            ps = psC.tile([P, NB], F32, tag="ps")
                            for d2 in range(D2):
                                nc.tensor.matmul(
                                    ps[:],
                                    lhsT=xo8[d2][:, :, kt * P:(kt + 1) * P],
                                    rhs=wv8[d2][:, :, half * NB:(half + 1) * NB],
                                    start=(d2 == 0),
                                    stop=(d2 == D2 - 1),
                                    perf_mode=DR,
                                )
                            nc.vector.tensor_add(
                                vDR[kt // 2][:, kt % 2, half * 8:(half + 1) * 8, 0:HD],
                                ps[:].rearrange("p (h j) -> p h j", j=HD),
                                bv_rep[:, half * NB:(half + 1) * NB].rearrange(
                                    "p (h j) -> p h j", j=HD
                                ),
                            )
                    for k2 in range(KT // 2):
                        nc.vector.memset(vDR[k2][:, :, :, HD:HD + 1], 1.0)

                # ---- feeders: K[f], Q[f], scores(f, 0); V after 3 lead units;
                # quads drain the exp'd probabilities as they land ----
                for f in range(FT):
                    emit_kq(f)
                    emit_scores(f, 0)
                    if f % 2 == 1 and f >= 3:
                        emit_quad((f - 3) // 2, 0)
                    if f == 2:
                        emit_v()
                        wv_es.close()
                emit_quad(3, 0)
            proj_es.close()

            # projection inputs + psC freed; load O-proj weights + LN consts
            with (
                tc.tile_pool(name="late", bufs=1) as latep,
                tc.tile_pool(name="stE", bufs=3) as stE,
                tc.tile_pool(name="psE", bufs=2, space="PSUM") as psE,
            ):
                wout8t = []
                for f2 in range(D2):
                    wo = latep.tile([P, 2, DIM], F8, name=f"wo{f2}")
                    nc.gpsimd.dma_start(
                        out=wo[:],
                        in_=wout8[f2 * P:(f2 + 1) * P, :].rearrange(
                            "p (o n) -> p o n", o=2),
                    )
                    wout8t.append(wo)
                bout_rep = latep.tile([P, DIM], F32)
                nc.gpsimd.dma_start(out=bout_rep[:], in_=bout[:].to_broadcast([P, DIM]))
                gamma_rep = latep.tile([P, DIM], F32)
                nc.gpsimd.dma_start(out=gamma_rep[:], in_=gamma[:].to_broadcast([P, DIM]))
                beta_rep = latep.tile([P, DIM], F32)
                nc.gpsimd.dma_start(out=beta_rep[:], in_=beta[:].to_broadcast([P, DIM]))

                def emit_owave(w):
                    """Out-projection + residual + LayerNorm for token tiles
                    4w..4w+3 (query chunk w)."""
                    for t in range(4 * w, 4 * w + 4):
                        tsl = slice(t * P, (t + 1) * P)
                        x32 = stE.tile([P, DIM], F32, tag="xr")
                        nc.gpsimd.dma_start(out=x32[:], in_=x_own[tsl, :])
                        y_sb = stE.tile([P, DIM], F32, tag="ysb")
                        for half in range(2):
                            ps = psE.tile([P, NB], F32, tag="ps")
                            for f2 in range(D2):
                                nc.tensor.matmul(
                                    ps[:],
                                    lhsT=attn_sb[f2][:, :, tsl],
                                    rhs=wout8t[f2][:, :, half * NB:(half + 1) * NB],
                                    start=(f2 == 0),
                                    stop=(f2 == D2 - 1),
                                    perf_mode=DR,
                                )
                            nc.vector.tensor_add(
                                y_sb[:, half * NB:(half + 1) * NB],
                                ps[:],
                                x32[:, half * NB:(half + 1) * NB],
                            )
                        nc.vector.tensor_add(y_sb[:], y_sb[:], bout_rep[:])
                        # LayerNorm over the 1024 free dim
                        st = stE.tile([P, 2, 6], F32, tag="bn")
                        nc.vector.bn_stats(st[:, 0, :], y_sb[:, 0:NB])
                        nc.vector.bn_stats(st[:, 1, :], y_sb[:, NB:DIM])
                        mv = stE.tile([P, 2], F32, tag="mv")
                        nc.vector.bn_aggr(mv[:], st[:])
                        nm = stE.tile([P, 1], F32, tag="nm")
                        nc.vector.tensor_scalar_mul(nm[:], mv[:, 0:1], -1.0)
                        rstd = stE.tile([P, 1], F32, tag="rstd")
                        nc.scalar.activation(
                            rstd[:], mv[:, 1:2], SQRT, bias=eps_t[:], scale=1.0
                        )
                        nc.vector.reciprocal(rstd[:], rstd[:])
                        yn = stE.tile([P, DIM], F32, tag="yn")
                        nc.vector.tensor_scalar(
                            out=yn[:], in0=y_sb[:], scalar1=nm[:], scalar2=rstd[:],
                            op0=ADD, op1=MULT,
                        )
                        nc.vector.tensor_mul(yn[:], yn[:], gamma_rep[:])
                        nc.vector.tensor_add(yn[:], yn[:], beta_rep[:])
                        nc.sync.dma_start(out=y_ext[tsl, :], in_=yn[:])

                if debug:
                    nc.sync.dma_start(out=dbg["kT0"][:], in_=kTs[0][:])
                    nc.sync.dma_start(out=dbg["qT0"][:], in_=qTs[0][:])
                    nc.sync.dma_start(
                        out=dbg["vS0"][:],
                        in_=vDR[0][:].rearrange("p o h j -> p (o h j)"),
                    )
                    # (placed with other taps; vDR includes the ones column)
                    nc.sync.dma_start(
                        out=dbg["attn0"][:],
                        in_=attn_sb[0][:].rearrange("p o t -> p (o t)"),
                    )
                # q=1 attention; out-projections emitted last so they fill PE
                # idle without outranking the score feeders
                for c in range(4):
                    emit_scores(2 * c, 1)
                    emit_scores(2 * c + 1, 1)
                    emit_quad(c, 1)
                emit_owave(0)
                emit_owave(1)

    if split_waits:
        _split_sync_waits(nc)
    return nc


_NC_CACHE = None
LAST_RESULT = None


def _get_nc():
    global _NC_CACHE
    if _NC_CACHE is None:
        _NC_CACHE = _build_bass()
    return _NC_CACHE


def _pair_rows(m8):
    """[1024, C] fp8 -> [512, 2C]: row dt2*128+i holds source rows
    {dt2*256+i, dt2*256+128+i} side by side (DoubleRow contraction pairs)."""
    c = m8.shape[1]
    return np.ascontiguousarray(
        m8.reshape(4, 2, P, c).swapaxes(1, 2).reshape(DIM // 2, 2 * c)
    )


def kernel(embedding1, embedding2, Wqkv, bqkv, Wout, bout, gamma, beta):
    global LAST_RESULT
    import ml_dtypes

    F8NP = ml_dtypes.float8_e4m3
    embs = [np.ascontiguousarray(np.asarray(embedding1, dtype=np.float32)),
            np.ascontiguousarray(np.asarray(embedding2, dtype=np.float32))]
    w = np.asarray(Wqkv, dtype=np.float32)
    w8 = np.concatenate(
        [_pair_rows(w[:, s * DIM:(s + 1) * DIM].astype(F8NP)) for s in range(3)],
        axis=1,
    )
    wo8 = _pair_rows(np.asarray(Wout, dtype=np.float32).astype(F8NP))
    bq = np.ascontiguousarray(np.asarray(bqkv, dtype=np.float32)).reshape(3 * DIM)
    bo = np.ascontiguousarray(np.asarray(bout, dtype=np.float32)).reshape(1, DIM)
    ga = np.ascontiguousarray(np.asarray(gamma, dtype=np.float32)).reshape(1, DIM)
    be = np.ascontiguousarray(np.asarray(beta, dtype=np.float32)).reshape(1, DIM)
    # host-side layout prep: fp8 cast + transpose + DoubleRow row pairing
    xT = [[_pair_rows(np.ascontiguousarray(embs[s][b].astype(F8NP).T))
           for b in range(2)] for s in range(2)]  # each [DIM//2, 2*N_TOK]

    nc = _get_nc()
    in_maps = []
    layout = []  # (s, b, c) per core
    for s in range(2):
        for b in range(2):
            for c in range(2):
                xo = xT[s][b].reshape(DIM // 2, 2, N_TOK)
                in_maps.append({
                    "x_own": np.ascontiguousarray(embs[s][b, c * NQ:(c + 1) * NQ, :]),
                    "xT8": np.ascontiguousarray(
                        xo[:, :, c * NQ:(c + 1) * NQ].reshape(DIM // 2, 2 * NQ)),
                    "xoT8": xT[1 - s][b],
                    "wqkv8": w8,
                    "wout8": wo8,
                    "bqkv": bq,
                    "bout": bo,
                    "gamma": ga,
                    "beta": be,
                })
                layout.append((s, b, c))

    trace = os.environ.get("TRN_KERNEL_TRACE", "") not in ("", "0")
    if trace:
        _ensure_ntff_hook()
        _patch_upload_artifacts()
    res = run_bass_kernel_spmd(
        nc, in_maps, core_ids=list(range(8)), trace=trace,
    )
    LAST_RESULT = res

    out = np.zeros((2, 2, N_TOK, DIM), dtype=np.float32)
    for i, (s, b, c) in enumerate(layout):
        out[s, b, c * NQ:(c + 1) * NQ, :] = np.asarray(res.results[i]["y"])
    return out
